# revision 29
# baseline (speedup 1.0000x reference)
"""CxAM (context attention module) Trainium2 Bass kernel.

Full-input contract: kernel(**inputs) takes the unsharded tensors from
setup_inputs() and returns the full [16, 256, 64, 64] fp32 output.

Math (per sample, X = x[b] reshaped [C, H*W]):
    v      = Wv @ X + bv
    k_mean = mean_p(Wk @ X + bk) = Wk @ mean_p(X) + bk     (mean commutes)
    att    = sigmoid((Wq^T k_mean)^T X + bq.k_mean)        (Q path collapses)
    out    = v * att[None, :]

Distribution: data-parallel over batch, 2 samples per NeuronCore x 8 cores.

The kernel is HBM-bound (per core: x in + out out), so x is shipped as
bf16 — half the input bytes — and all large matmuls run in bf16 at the
same 1 cycle/row PE rate as fp32r. bf16 rounding of x and the weights
gives ~3e-3 element-relative error, far inside the 2e-2 gate; the
elementwise epilogue (bias, sigmoid, multiply) stays fp32.

Device strategy per core:
  - single-pass bf16 V projection accumulating CCH chunks in PSUM; the
    Tensor engine carries only V (16384 cyc) + logit (8192 cyc) per sample
  - k_mean via DVE pixel half-sum reduces + a tiny fp32r Wk matmul (keeps
    the 8192-cycle K projection off the Tensor engine)
  - w_eff = Wq^T k_mean via tiny stacked matmuls (N=2 for the fp32r
    even-moving-dim ISA restriction); broadcast along the free dim with a
    tensor_scalar against a ones tile; c = bq.k_mean computed replicated
    over partitions by a matmul against a host-tiled bq (no gpsimd
    partition_broadcast, keeping the Pool engine free)
  - logit matmul uses the broadcast lhsT so PSUM comes out replicated over
    all 128 partitions; sigmoid+bias on ScalarE straight out of PSUM
  - epilogue split across engines: o=0 v-bias+att-mul on VectorE, o=1
    v-bias on ScalarE + att-mul on GpSimd
  - DMA split across all three queues: x loads on the SP HWDGE (one
    8KB-line DMA per x tile), o=0 stores on the Activation HWDGE, o=1
    stores on the GpSimd software DGE (one 16KB-line DMA per half; the
    three queues run concurrently, and per-descriptor costs -- ~46ns on
    the software DGE -- would cap a single-queue kernel)
"""

import sys

sys.path.insert(0, "/opt/trn_rl_repo")

from contextlib import ExitStack

import ml_dtypes
import numpy as np

import concourse.mybir as mybir
import concourse.tile as tile
from concourse import bacc
from concourse.bass_utils import run_bass_kernel_spmd

F32 = mybir.dt.float32
F32R = mybir.dt.float32r
BF16 = mybir.dt.bfloat16
AF = mybir.ActivationFunctionType
ALU = mybir.AluOpType

B, C, H, W = 16, 256, 64, 64
HW = H * W
CR = 32
N_CORES = 8
BPC = B // N_CORES
NCH = 512
NP = HW // NCH
CCH = C // 128

_CACHED_NC = None


def _build(rep=1):
    nc = bacc.Bacc("TRN2", target_bir_lowering=False, debug=False,
                   num_devices=N_CORES)

    x_d = nc.dram_tensor("x", [BPC * C, HW], BF16, kind="ExternalInput").ap()
    out_d = nc.dram_tensor("out", [BPC * C, HW], F32, kind="ExternalOutput").ap()
    wv_d = nc.dram_tensor("wvT16", [C, C], BF16, kind="ExternalInput").ap()
    wk_d = nc.dram_tensor("wkT32", [C, CR], F32, kind="ExternalInput").ap()
    wq_d = nc.dram_tensor("wq32", [CR, C], F32R, kind="ExternalInput").ap()
    bqb_d = nc.dram_tensor("bqb32", [CR, 128], F32R, kind="ExternalInput").ap()
    bkq_d = nc.dram_tensor("bk32", [CR, 1], F32, kind="ExternalInput").ap()
    bv_d = nc.dram_tensor("bv2", [128, 2], F32, kind="ExternalInput").ap()

    with tile.TileContext(nc) as tc, ExitStack() as ctx:
        consts = ctx.enter_context(tc.tile_pool(name="consts", bufs=1))
        xin = ctx.enter_context(tc.tile_pool(name="xin", bufs=5))
        attp = ctx.enter_context(tc.tile_pool(name="att", bufs=3))
        outp = ctx.enter_context(tc.tile_pool(name="outp", bufs=3))
        vsb = ctx.enter_context(tc.tile_pool(name="vsb", bufs=6))
        small = ctx.enter_context(tc.tile_pool(name="small", bufs=4))
        pv = ctx.enter_context(tc.tile_pool(name="pv", bufs=4, space="PSUM"))
        pl = ctx.enter_context(tc.tile_pool(name="pl", bufs=2, space="PSUM"))
        pk = ctx.enter_context(tc.tile_pool(name="pk", bufs=1, space="PSUM"))
        pw = ctx.enter_context(tc.tile_pool(name="pw", bufs=1, space="PSUM"))

        wv = [consts.tile([128, C], BF16, tag=f"wv{i}", name=f"wv{i}")
              for i in range(CCH)]
        wk = [consts.tile([128, CR], F32, tag=f"wk{i}", name=f"wk{i}")
              for i in range(CCH)]
        for cc in range(CCH):
            nc.sync.dma_start(wv[cc][:], wv_d[cc * 128:(cc + 1) * 128, :])
            nc.sync.dma_start(wk[cc][:], wk_d[cc * 128:(cc + 1) * 128, :])
        wq = consts.tile([CR, C], F32R, tag="wq")
        nc.sync.dma_start(wq[:], wq_d[:])
        bqb = consts.tile([CR, 128], F32R, tag="bqb")
        nc.sync.dma_start(bqb[:], bqb_d[:])
        bkq = consts.tile([CR, 1], F32, tag="bkq")
        nc.sync.dma_start(bkq[:], bkq_d[:])
        bv = consts.tile([128, 2], F32, tag="bv")
        nc.sync.dma_start(bv[:], bv_d[:])
        ones = consts.tile([128, 128], F32, tag="ones")
        nc.vector.memset(ones[:], 1.0)

        for s in range(BPC * rep):
            # ---- load x (split for earlier compute start) ----
            xt = [xin.tile([128, HW], BF16, tag="x", name=f"xt{s}_{i}")
                  for i in range(CCH)]
            for cc in range(CCH):
                base = (s % BPC) * C + cc * 128
                nc.sync.dma_start(xt[cc][:], x_d[base:base + 128, :])

            # ---- k_mean: DVE pixel half-sums, then a tiny Wk matmul ----
            # (keeps the 8192-cycle K projection off the Tensor engine)
            xsum = [small.tile([128, 2], F32, tag=f"xsum{cc}",
                               name=f"xsum{s}_{cc}") for cc in range(CCH)]
            for cc in range(CCH):
                for h in range(2):
                    nc.vector.reduce_sum(
                        xsum[cc][:, h:h + 1],
                        xt[cc][:, h * (HW // 2):(h + 1) * (HW // 2)],
                        axis=mybir.AxisListType.X)
            pkt = pk.tile([CR, 2], F32, tag="pk", name=f"pk{s}")
            for cc in range(CCH):
                nc.tensor.matmul(pkt[:], wk[cc][:], xsum[cc][:],
                                 start=(cc == 0), stop=(cc == CCH - 1))
            # add the two half-sums + t = s/HW + bk (2 cols: fp32r even N)
            sk = small.tile([CR, 1], F32, tag="sk", name=f"sk{s}")
            nc.vector.reduce_sum(sk[:], pkt[:], axis=mybir.AxisListType.X)
            tsb = small.tile([CR, 2], F32R, tag="tsb", name=f"tsb{s}")
            nc.vector.tensor_scalar(tsb[:], sk[:].broadcast_to([CR, 2]),
                                    1.0 / HW, bkq[:], ALU.mult, ALU.add)

            # ---- w_eff (+ c) ----
            pwt = pw.tile([128, NCH], F32, tag="pw", name=f"pw{s}")
            for ct in range(CCH):
                nc.tensor.matmul(pwt[:, 2 * ct:2 * ct + 2],
                                 wq[:, ct * 128:(ct + 1) * 128],
                                 tsb[:], start=True, stop=True)
            nc.tensor.matmul(pwt[:, 4:6], bqb[:], tsb[:],
                             start=True, stop=True)

            weff = [small.tile([128, 128], BF16, tag=f"weff{ct}",
                               name=f"weff{s}_{ct}") for ct in range(CCH)]
            for ct in range(CCH):
                nc.vector.tensor_scalar(weff[ct][:], ones[:],
                                        pwt[:, 2 * ct:2 * ct + 1], None,
                                        ALU.mult)
            crep = small.tile([128, 1], F32, tag="crep", name=f"crep{s}")
            nc.vector.tensor_copy(crep[:], pwt[:, 4:5])

            # ---- logit (replicated over partitions) + sigmoid ----
            # p processed in pairs with ct outer, so each weff[ct] weight
            # load on the PE covers 2 matmuls instead of 1
            att = attp.tile([128, HW], F32, tag="att", name=f"att{s}")
            for pp in range(NP // 2):
                plts = [pl.tile([128, NCH], F32, tag="pl",
                                name=f"pl{s}_{2 * pp + i}") for i in range(2)]
                for ct in range(CCH):
                    for i in range(2):
                        p = 2 * pp + i
                        nc.tensor.matmul(plts[i][:], weff[ct][:],
                                         xt[ct][:, p * NCH:(p + 1) * NCH],
                                         start=(ct == 0), stop=(ct == CCH - 1))
                for i in range(2):
                    p = 2 * pp + i
                    nc.scalar.activation(att[:, p * NCH:(p + 1) * NCH],
                                         plts[i][:], AF.Sigmoid, bias=crep[:])

            # ---- V projection (single-pass bf16) + bias + att mul + store ----
            # epilogue split across engines: o=0 bias+mul on DVE, o=1 bias on
            # ScalarE and mul on GpSimd, so no single engine carries the full
            # 2x[128,HW] elementwise load
            ots = []
            for o in range(2):
                ot = outp.tile([128, HW], F32, tag="ot", name=f"ot{s}_{o}")
                ots.append(ot)
                for pg in range(NP // 4):
                    pvts = [pv.tile([128, NCH], F32, tag="pv",
                                    name=f"pv{s}_{o}_{4 * pg + i}")
                            for i in range(4)]
                    for cc in range(CCH):
                        for i in range(4):
                            p = 4 * pg + i
                            nc.tensor.matmul(
                                pvts[i][:], wv[cc][:, o * 128:(o + 1) * 128],
                                xt[cc][:, p * NCH:(p + 1) * NCH],
                                start=(cc == 0), stop=(cc == CCH - 1))
                    for i in range(4):
                        p = 4 * pg + i
                        vt = vsb.tile([128, NCH], F32, tag="vt",
                                      name=f"vt{s}_{o}_{p}")
                        if o == 0:
                            nc.vector.tensor_scalar(vt[:], pvts[i][:], 1.0,
                                                    bv[:, 0:1],
                                                    ALU.mult, ALU.add)
                            nc.vector.tensor_mul(
                                ot[:, p * NCH:(p + 1) * NCH], vt[:],
                                att[:, p * NCH:(p + 1) * NCH])
                        else:
                            nc.scalar.activation(vt[:], pvts[i][:],
                                                 AF.Identity, bias=bv[:, 1:2])
                            nc.gpsimd.tensor_mul(
                                ot[:, p * NCH:(p + 1) * NCH], vt[:],
                                att[:, p * NCH:(p + 1) * NCH])
            base = (s % BPC) * C
            nc.gpsimd.dma_start(out_d[base + 128:base + 256, :], ots[1][:])
            nc.scalar.dma_start(out_d[base:base + 128, :], ots[0][:])

    nc.compile()
    return nc


def _host_prep(Wq, bq, Wk, bk, Wv, bv):
    Wq = np.asarray(Wq, np.float32)
    bq = np.asarray(bq, np.float32)
    Wk = np.asarray(Wk, np.float32)
    bk = np.asarray(bk, np.float32)
    Wv = np.asarray(Wv, np.float32)
    bv = np.asarray(bv, np.float32)
    return {
        "wvT16": np.ascontiguousarray(Wv.T.astype(ml_dtypes.bfloat16)),
        "wkT32": np.ascontiguousarray(Wk.T),
        "wq32": np.ascontiguousarray(Wq),
        "bqb32": np.ascontiguousarray(np.tile(bq[:, None], (1, 128))),
        "bk32": np.ascontiguousarray(bk[:, None]),
        "bv2": np.ascontiguousarray(bv.reshape(2, 128).T),
    }


def _prep_x(x):
    return np.asarray(x, np.float32).reshape(B, C, HW).astype(ml_dtypes.bfloat16)


def kernel(x, Wq, bq, Wk, bk, Wv, bv):
    global _CACHED_NC
    if _CACHED_NC is None:
        _CACHED_NC = _build()
    nc = _CACHED_NC

    prep = _host_prep(Wq, bq, Wk, bk, Wv, bv)
    x = _prep_x(x)
    in_maps = []
    for core in range(N_CORES):
        m = {"x": np.ascontiguousarray(
            x[core * BPC:(core + 1) * BPC].reshape(BPC * C, HW))}
        m.update(prep)
        in_maps.append(m)

    res = run_bass_kernel_spmd(nc, in_maps, core_ids=list(range(N_CORES)))

    out = np.empty((B, C, HW), np.float32)
    for core in range(N_CORES):
        out[core * BPC:(core + 1) * BPC] = \
            res.results[core]["out"].reshape(BPC, C, HW)
    return out.reshape(B, C, H, W)


# revision 31
# speedup vs baseline: 1.4031x; 1.4031x over previous
"""CxAM (context attention module) Trainium2 Bass kernel.

Full-input contract: kernel(**inputs) takes the unsharded tensors from
setup_inputs() and returns the full [16, 256, 64, 64] fp32 output.

Math (per sample, X = x[b] reshaped [C, H*W]):
    v      = Wv @ X + bv
    k_mean = mean_p(Wk @ X + bk) = Wk @ mean_p(X) + bk     (mean commutes)
    att    = sigmoid((Wq^T k_mean)^T X + bq.k_mean)        (Q path collapses)
    out    = v * att[None, :]

The Wq/Wk/bq/bk algebra is folded on the HOST into
    MT = Wk^T Wq   [C,C],  r = bq Wk  [C],  cq = bk Wq  [C],  c1 = bq.bk
so the device-side attention chain is just
    w_eff = (MT^T xsum)/HW + cq,   c = (r.xsum)/HW + c1
fed directly by DVE pixel sums of x -- one tiny PE matmul group with no
intermediate DVE hops on the Tensor engine's critical path.

Distribution: data-parallel over batch, 2 samples per NeuronCore x 8 cores.

x is shipped as bf16 (halves input HBM bytes; bf16 matmul runs at the
same 1 cycle/row PE rate as fp32r; ~2e-3 scale-relative error vs the
2e-2 gate). The elementwise epilogue stays fp32.

Per-core schedule (software-pipelined one sample ahead so the PE never
stalls mid-stream):
    body(s): issue xt(s+2) loads | logit(s)+sigmoid | w-chain(s+1)
             | V(s) + bias + att-mul | stores(s) | xsum(s+2) reduces
  - PE stream: logit(s) x16, w-chain(s+1) x6 (tiny), V(s) x32, ...
    w-chain needs only xsum(s+1) (ready since body(s-1)); logit(s+1)
    needs weff(s+1), computed by the DVE early in the V(s) window
  - epilogue split: o=0 v-bias+att-mul on VectorE, o=1 v-bias on ScalarE
    + att-mul on GpSimd
  - DMA split across all three queues: x loads on the SP HWDGE (one
    8KB-line DMA per x tile), o=0 stores on the Activation HWDGE, o=1
    stores on the GpSimd software DGE (one 16KB-line DMA per half)
"""

import sys

sys.path.insert(0, "/opt/trn_rl_repo")

from contextlib import ExitStack

import ml_dtypes
import numpy as np

import concourse.mybir as mybir
import concourse.tile as tile
from concourse import bacc
from concourse.bass_utils import run_bass_kernel_spmd

F32 = mybir.dt.float32
F32R = mybir.dt.float32r
BF16 = mybir.dt.bfloat16
AF = mybir.ActivationFunctionType
ALU = mybir.AluOpType

B, C, H, W = 16, 256, 64, 64
HW = H * W
CR = 32
N_CORES = 8
BPC = B // N_CORES
NCH = 512
NP = HW // NCH
CCH = C // 128

_CACHED_NC = None


def _build(rep=1):
    nc = bacc.Bacc("TRN2", target_bir_lowering=False, debug=False,
                   num_devices=N_CORES)

    x_d = nc.dram_tensor("x", [BPC * C, HW], BF16, kind="ExternalInput").ap()
    out_d = nc.dram_tensor("out", [BPC * C, HW], F32, kind="ExternalOutput").ap()
    wv_d = nc.dram_tensor("wvT16", [C, C], BF16, kind="ExternalInput").ap()
    mt_d = nc.dram_tensor("mt32", [C, C], F32, kind="ExternalInput").ap()
    rr_d = nc.dram_tensor("rrep32", [C, 128], F32, kind="ExternalInput").ap()
    cq_d = nc.dram_tensor("cq32", [C, 1], F32, kind="ExternalInput").ap()
    c1_d = nc.dram_tensor("c132", [128, 1], F32, kind="ExternalInput").ap()
    bv_d = nc.dram_tensor("bv2", [128, 2], F32, kind="ExternalInput").ap()

    with tile.TileContext(nc) as tc, ExitStack() as ctx:
        consts = ctx.enter_context(tc.tile_pool(name="consts", bufs=1))
        xin = ctx.enter_context(tc.tile_pool(name="xin", bufs=7))
        attp = ctx.enter_context(tc.tile_pool(name="att", bufs=3))
        outp = ctx.enter_context(tc.tile_pool(name="outp", bufs=3))
        vsb = ctx.enter_context(tc.tile_pool(name="vsb", bufs=6))
        small = ctx.enter_context(tc.tile_pool(name="small", bufs=4))
        pv = ctx.enter_context(tc.tile_pool(name="pv", bufs=4, space="PSUM"))
        pl = ctx.enter_context(tc.tile_pool(name="pl", bufs=2, space="PSUM"))
        pw = ctx.enter_context(tc.tile_pool(name="pw", bufs=1, space="PSUM"))

        wv = [consts.tile([128, C], BF16, tag=f"wv{i}", name=f"wv{i}")
              for i in range(CCH)]
        mt = [consts.tile([128, C], F32, tag=f"mt{i}", name=f"mt{i}")
              for i in range(CCH)]
        rr = [consts.tile([128, 128], F32, tag=f"rr{i}", name=f"rr{i}")
              for i in range(CCH)]
        cq = [consts.tile([128, 1], F32, tag=f"cq{i}", name=f"cq{i}")
              for i in range(CCH)]
        for cc in range(CCH):
            nc.sync.dma_start(wv[cc][:], wv_d[cc * 128:(cc + 1) * 128, :])
            nc.sync.dma_start(mt[cc][:], mt_d[cc * 128:(cc + 1) * 128, :])
            nc.sync.dma_start(rr[cc][:], rr_d[cc * 128:(cc + 1) * 128, :])
            nc.sync.dma_start(cq[cc][:], cq_d[cc * 128:(cc + 1) * 128, :])
        c1t = consts.tile([128, 1], F32, tag="c1t")
        nc.sync.dma_start(c1t[:], c1_d[:])
        bv = consts.tile([128, 2], F32, tag="bv")
        nc.sync.dma_start(bv[:], bv_d[:])

        NS = BPC * rep

        def emit_xt(s):
            xt = [xin.tile([128, HW], BF16, tag="x", name=f"xt{s}_{i}")
                  for i in range(CCH)]
            for cc in range(CCH):
                base = (s % BPC) * C + cc * 128
                nc.sync.dma_start(xt[cc][:], x_d[base:base + 128, :])
            return xt

        def emit_xsum(s, xt):
            xs = [small.tile([128, 2], F32, tag=f"xsum{cc}",
                             name=f"xsum{s}_{cc}") for cc in range(CCH)]
            for cc in range(CCH):
                for h in range(2):
                    nc.vector.reduce_sum(
                        xs[cc][:, h:h + 1],
                        xt[cc][:, h * (HW // 2):(h + 1) * (HW // 2)],
                        axis=mybir.AxisListType.X)
            return xs

        def emit_wchain(s, xs):
            # pwt cols 0:2 = MT^T[ct0] xsum, 2:4 = MT^T[ct1] xsum,
            # 4:6 = r.xsum replicated over partitions (sequential PSUM
            # groups, each accumulating over cc)
            pwt = pw.tile([128, 6], F32, tag="pw", name=f"pw{s}")
            for ct in range(CCH):
                for cc in range(CCH):
                    nc.tensor.matmul(pwt[:, 2 * ct:2 * ct + 2],
                                     mt[cc][:, ct * 128:(ct + 1) * 128],
                                     xs[cc][:],
                                     start=(cc == 0), stop=(cc == CCH - 1))
            for cc in range(CCH):
                nc.tensor.matmul(pwt[:, 4:6], rr[cc][:], xs[cc][:],
                                 start=(cc == 0), stop=(cc == CCH - 1))
            # DVE: add half-sum columns, then scale 1/HW + bias, broadcast
            weff = []
            for ct in range(CCH):
                tmp = small.tile([128, 1], F32, tag=f"tmp{ct}",
                                 name=f"tmp{s}_{ct}")
                nc.vector.tensor_scalar(tmp[:], pwt[:, 2 * ct:2 * ct + 1],
                                        1.0, pwt[:, 2 * ct + 1:2 * ct + 2],
                                        ALU.mult, ALU.add)
                wt = small.tile([128, 128], BF16, tag=f"weff{ct}",
                                name=f"weff{s}_{ct}")
                nc.vector.tensor_scalar(wt[:], tmp[:].broadcast_to([128, 128]),
                                        1.0 / HW, cq[ct][:],
                                        ALU.mult, ALU.add)
                weff.append(wt)
            tmpc = small.tile([128, 1], F32, tag="tmpc", name=f"tmpc{s}")
            nc.vector.tensor_scalar(tmpc[:], pwt[:, 4:5], 1.0, pwt[:, 5:6],
                                    ALU.mult, ALU.add)
            crep = small.tile([128, 1], F32, tag="crep", name=f"crep{s}")
            nc.vector.tensor_scalar(crep[:], tmpc[:], 1.0 / HW, c1t[:],
                                    ALU.mult, ALU.add)
            return weff, crep

        # prologue: two samples of x + xsum, w-chain for sample 0
        xts = {0: emit_xt(0)}
        if NS > 1:
            xts[1] = emit_xt(1)
        xsums = {0: emit_xsum(0, xts[0])}
        if NS > 1:
            xsums[1] = emit_xsum(1, xts[1])
        weff, crep = emit_wchain(0, xsums[0])

        for s in range(NS):
            xt = xts.pop(s)
            if s + 2 < NS:
                xts[s + 2] = emit_xt(s + 2)

            # ---- logit (replicated over partitions) + sigmoid ----
            att = attp.tile([128, HW], F32, tag="att", name=f"att{s}")
            for p in range(NP):
                plt = pl.tile([128, NCH], F32, tag="pl", name=f"pl{s}_{p}")
                for ct in range(CCH):
                    nc.tensor.matmul(plt[:], weff[ct][:],
                                     xt[ct][:, p * NCH:(p + 1) * NCH],
                                     start=(ct == 0), stop=(ct == CCH - 1))
                nc.scalar.activation(att[:, p * NCH:(p + 1) * NCH], plt[:],
                                     AF.Sigmoid, bias=crep[:])

            # ---- next sample's attention-weight chain (tiny PE matmuls
            # fed by precomputed xsum; weff ready before logit(s+1)) ----
            if s + 1 < NS:
                weff, crep = emit_wchain(s + 1, xsums[s + 1])

            # ---- V projection (single-pass bf16) + bias + att mul ----
            ots = []
            for o in range(2):
                ot = outp.tile([128, HW], F32, tag="ot", name=f"ot{s}_{o}")
                ots.append(ot)
                for p in range(NP):
                    pvt = pv.tile([128, NCH], F32, tag="pv", name=f"pv{s}_{o}_{p}")
                    for cc in range(CCH):
                        nc.tensor.matmul(
                            pvt[:], wv[cc][:, o * 128:(o + 1) * 128],
                            xt[cc][:, p * NCH:(p + 1) * NCH],
                            start=(cc == 0), stop=(cc == CCH - 1))
                    vt = vsb.tile([128, NCH], F32, tag="vt",
                                  name=f"vt{s}_{o}_{p}")
                    if o == 0:
                        nc.vector.tensor_scalar(vt[:], pvt[:], 1.0,
                                                bv[:, 0:1], ALU.mult, ALU.add)
                        nc.vector.tensor_mul(ot[:, p * NCH:(p + 1) * NCH],
                                             vt[:],
                                             att[:, p * NCH:(p + 1) * NCH])
                    else:
                        nc.scalar.activation(vt[:], pvt[:], AF.Identity,
                                             bias=bv[:, 1:2])
                        nc.gpsimd.tensor_mul(ot[:, p * NCH:(p + 1) * NCH],
                                             vt[:],
                                             att[:, p * NCH:(p + 1) * NCH])
            base = (s % BPC) * C
            nc.gpsimd.dma_start(out_d[base + 128:base + 256, :], ots[1][:])
            nc.scalar.dma_start(out_d[base:base + 128, :], ots[0][:])

            # ---- xsum for sample s+2 (xt issued at body start; the DMA
            # has the whole body to land) ----
            xsums.pop(s)
            if s + 2 < NS:
                xsums[s + 2] = emit_xsum(s + 2, xts[s + 2])

    nc.compile()
    return nc


def _host_prep(Wq, bq, Wk, bk, Wv, bv):
    Wq = np.asarray(Wq, np.float32)
    bq = np.asarray(bq, np.float32)
    Wk = np.asarray(Wk, np.float32)
    bk = np.asarray(bk, np.float32)
    Wv = np.asarray(Wv, np.float32)
    bv = np.asarray(bv, np.float32)
    MT = (Wk.T @ Wq).astype(np.float32)          # [C k, C c]
    r = (bq @ Wk).astype(np.float32)             # [C]
    cqv = (bk @ Wq).astype(np.float32)           # [C]
    c1 = np.float32(bq @ bk)
    return {
        "wvT16": np.ascontiguousarray(Wv.T.astype(ml_dtypes.bfloat16)),
        "mt32": np.ascontiguousarray(MT),
        "rrep32": np.ascontiguousarray(np.tile(r[:, None], (1, 128))),
        "cq32": np.ascontiguousarray(cqv[:, None]),
        "c132": np.full((128, 1), c1, np.float32),
        "bv2": np.ascontiguousarray(bv.reshape(2, 128).T),
    }


def _prep_x(x):
    return np.asarray(x, np.float32).reshape(B, C, HW).astype(ml_dtypes.bfloat16)


def kernel(x, Wq, bq, Wk, bk, Wv, bv):
    global _CACHED_NC
    if _CACHED_NC is None:
        _CACHED_NC = _build()
    nc = _CACHED_NC

    prep = _host_prep(Wq, bq, Wk, bk, Wv, bv)
    x = _prep_x(x)
    in_maps = []
    for core in range(N_CORES):
        m = {"x": np.ascontiguousarray(
            x[core * BPC:(core + 1) * BPC].reshape(BPC * C, HW))}
        m.update(prep)
        in_maps.append(m)

    res = run_bass_kernel_spmd(nc, in_maps, core_ids=list(range(N_CORES)))

    out = np.empty((B, C, HW), np.float32)
    for core in range(N_CORES):
        out[core * BPC:(core + 1) * BPC] = \
            res.results[core]["out"].reshape(BPC, C, HW)
    return out.reshape(B, C, H, W)
